# revision 46
# baseline (speedup 1.0000x reference)
"""Trainium2 Bass kernel for nn_DeforConv_71605694759687.

ResBlock(stride2, 64->128) + DCNv2 (modulated deformable conv) + BN + ReLU.

Sharding (8 cores): (batch b = core//4, H-quarter q = core%4); each core
computes 32 output rows of out[b] end-to-end locally (halo via recompute,
no collectives).

DCNv2 is computed by TRUE bilinear gather: the Pool engine's ap_gather
fetches (x0, x0+1) feature pairs (d=2 on an x-duplicated fp16 field) at
per-pixel integer corner positions, for the two y corners of each of the
9 taps.  Corner weights mask*(1-fy)(1-fx) etc. are partition-broadcast
across the 64 channels of each deform group via tiny selector matmuls on
the PE, Hadamard-multiplied with the gathered pairs on the DVE, and the
9 taps x 4 corners = 36 terms are contracted on the PE (fp16).

Gather pixel order (per 2048-px pass): j = c*16 + r_loc*2 + qq so the
int16 index wrap (partition j%16) lowers to plain strided DMAs; the
permutation is absorbed by AP views and a host-side reshape.
"""

import numpy as np
import ml_dtypes
from contextlib import ExitStack

import concourse.bass as bass
import concourse.tile as tile
from concourse import mybir, bacc
from concourse.bass_utils import run_bass_kernel_spmd

F32 = mybir.dt.float32
F16 = mybir.dt.float16
BF16 = mybir.dt.bfloat16
I16 = mybir.dt.int16
I32 = mybir.dt.int32
AL = mybir.AluOpType
AF = mybir.ActivationFunctionType

P = 128
EPS = 1e-5
Ci, Co, DG, Cg = 64, 128, 2, 64
H, W = 128, 128          # output spatial (after stride-2)
QROWS = 32               # output rows per core
FR, FC = 38, 134         # F field: rows h0-3..h0+34, cols x in [-3,130]
F1R, F1C = 40, 130       # feat1: rows h0-4..h0+35, cols [-1,128]
XR, XC = 81, 258         # x_pad: rows 2*h0-9..2*h0+71, cols [-1,256]
NCHUNK = 1024
FLAT = FR * FC           # 5092
PASSW = 16 * FC          # 2144: flat elems per pass row-base
INW = 21 * FC + 132 + 1  # 2947: gather in-window elems per pass


def _h(x):
    return np.ascontiguousarray(np.asarray(x, dtype=np.float32).astype(np.float16))


def _b(x):
    return np.ascontiguousarray(
        np.asarray(x, dtype=np.float32).astype(ml_dtypes.bfloat16))


def _f(x):
    return np.ascontiguousarray(np.asarray(x, dtype=np.float32))


def build_nc():
    nc = bacc.Bacc(None)

    d_x = nc.dram_tensor("x_shard", [Ci, XR, XC], F16, kind="ExternalInput")
    d_l1 = nc.dram_tensor("lhsT1", [Ci, 9, P], F16, kind="ExternalInput")
    d_l2 = nc.dram_tensor("lhsT2", [P, 9, P], F16, kind="ExternalInput")
    d_lsc = nc.dram_tensor("lhsT_sc", [Ci, P], F16, kind="ExternalInput")
    d_loff = nc.dram_tensor("lhsT_off", [P, 9, 54], F16, kind="ExternalInput")
    d_ldcn = nc.dram_tensor("lhsT_dcn", [P, 9, P], F16, kind="ExternalInput")
    d_esel = nc.dram_tensor("e_sel", [P, 9, P], F16, kind="ExternalInput")
    d_cst = nc.dram_tensor("consts", [P, 8], F32, kind="ExternalInput")
    d_bq = nc.dram_tensor("bias_q", [P, 3], F32, kind="ExternalInput")
    d_by = nc.dram_tensor("base_y", [P, NCHUNK], F32, kind="ExternalInput")
    d_bx = nc.dram_tensor("base_x", [P, NCHUNK], F32, kind="ExternalInput")
    d_rm1 = nc.dram_tensor("rowmask1", [P, F1R], F32, kind="ExternalInput")
    d_rmf = nc.dram_tensor("rowmaskF", [P, FR], F32, kind="ExternalInput")
    # out layout: (pass, qq, m=c*8+r_loc) -> host re-maps
    d_out = nc.dram_tensor("out", [P, 2, 2, NCHUNK], F32, kind="ExternalOutput")
    # idx scratch, permuted [pass][t=2k+dd][r][qq][c] so each
    # (k,dd) wrap is one contiguous-source broadcast DMA
    d_scr = nc.dram_tensor("idx_scratch", [2, 18, 8, 2, 128], I16,
                           kind="Internal")

    with tile.TileContext(nc) as tc, ExitStack() as ctx:
        singles = ctx.enter_context(tc.tile_pool(name="singles", bufs=1))

        # ---- persistent SBUF ----
        # F field split per pass (x-pair duplicated): fd_a rows 0..23,
        # fd_b rows 16..37 -- so pass-0 gathers don't wait on late conv2
        fd_a = singles.tile([P, 24 * FC, 2], F16)
        fd_b = singles.tile([P, 22 * FC, 2], F16)
        ldcn = singles.tile([P, 9, P], F16)
        esel = singles.tile([P, 9, P], F16)
        cst = singles.tile([P, 8], F32)
        cw = singles.tile([P, 2, NCHUNK, 2], F16)   # corner wts (yc,px,xc)
        # one wrapped-idx tile per (pass, k): ISA gather deps are tracked
        # whole-tile, so per-gather tiles keep the dependency exact
        idxw_t = [[singles.tile([P, 256], I16, name=f"idxw{pp}_{k}")
                   for k in range(9)] for pp in range(2)]

        nc.sync.dma_start(out=ldcn[:], in_=d_ldcn[:])
        nc.sync.dma_start(out=esel[:], in_=d_esel[:])
        nc.sync.dma_start(out=cst[:], in_=d_cst[:])

        inv1, beta1 = cst[:, 0:1], cst[:, 1:2]
        inv2, beta2 = cst[:, 2:3], cst[:, 3:4]
        inv3, beta3 = cst[:, 4:5], cst[:, 5:6]

        fda4 = fd_a[:].rearrange("p (r c) e -> p r c e", r=24)
        fdb4 = fd_b[:].rearrange("p (r c) e -> p r c e", r=22)
        # zero the x-halo columns (cols 0..2 and 131..133 of every row)
        nc.vector.memset(fda4[:, :, 0:3, :], 0.0)
        nc.vector.memset(fda4[:, :, FC - 3:FC, :], 0.0)
        nc.vector.memset(fdb4[:, :, 0:3, :], 0.0)
        nc.vector.memset(fdb4[:, :, FC - 3:FC, :], 0.0)

        # ============ Phase A+B: ResBlock + offsets (wave-interleaved) ======
        # conv1/conv2/offset chunks are emitted interleaved so every engine
        # queue pipelines; the pass-0 offset->index chain is emitted mid-wave
        # so the first gather can start while conv2/offset chunks still run.
        ph_b = ctx.enter_context(tc.tile_pool(name="ph_b", bufs=1))
        pbt = ctx.enter_context(tc.tile_pool(name="ph_b_tmp", bufs=2))
        loff = ph_b.tile([P, 9, 54], F16)
        bq = ph_b.tile([P, 3], F32)
        basey = ph_b.tile([P, NCHUNK], F32)
        basex = ph_b.tile([P, NCHUNK], F32)
        q_t = ph_b.tile([P, 3, NCHUNK], F32)     # dy, dx, mm
        m_t = ph_b.tile([P, NCHUNK], F16)
        yf = ph_b.tile([P, NCHUNK], F32)
        xf = ph_b.tile([P, NCHUNK], F32)
        y0i = ph_b.tile([P, NCHUNK], I16)
        x0i = ph_b.tile([P, NCHUNK], I16)
        fy = ph_b.tile([P, NCHUNK], F16)
        fx = ph_b.tile([P, NCHUNK], F16)
        u0 = ph_b.tile([P, NCHUNK], F16)
        u1 = ph_b.tile([P, NCHUNK], F16)
        idx16 = ph_b.tile([P, NCHUNK], I16)
        # scratch aliases: dy/dx planes of q_t are dead once yf/xf exist,
        # and yf is dead once fy exists
        y0f = q_t[:, 0, :]
        x0f = q_t[:, 1, :]
        idxf = yf

        def phase_b_pass(pp):
            # elementwise chain on partition half [64pp, 64pp+64)
            # (dy/dx offset-conv biases are folded into base_y/base_x
            # host-side; the mask bias rides the sigmoid's bias port)
            s = slice(64 * pp, 64 * pp + 64)
            nc.scalar.activation(m_t[s], q_t[s, 2, :], AF.Sigmoid,
                                 bias=bq[s, 2:3])
            # sample coords (pass-relative row base baked into base_y)
            nc.vector.tensor_tensor(yf[s], q_t[s, 0, :], basey[s], AL.add)
            nc.vector.tensor_tensor(xf[s], q_t[s, 1, :], basex[s], AL.add)
            # floor via RNE(v - 0.5), clamped to the legal corner range
            nc.vector.tensor_scalar(y0i[s], yf[s], -0.5, None, AL.add)
            nc.vector.tensor_scalar(x0i[s], xf[s], -0.5, None, AL.add)
            nc.vector.tensor_scalar(y0i[s], y0i[s], 0, 20, AL.max, AL.min)
            nc.vector.tensor_scalar(x0i[s], x0i[s], 0, 132, AL.max, AL.min)
            nc.vector.tensor_copy(out=y0f[s], in_=y0i[s])
            nc.vector.tensor_copy(out=x0f[s], in_=x0i[s])
            # fractional parts, clamped to [0, 1]
            nc.vector.scalar_tensor_tensor(fy[s], y0f[s], -1.0, yf[s],
                                           AL.mult, AL.add)
            nc.vector.scalar_tensor_tensor(fx[s], x0f[s], -1.0, xf[s],
                                           AL.mult, AL.add)
            nc.vector.tensor_scalar(fy[s], fy[s], 0.0, 1.0, AL.max, AL.min)
            nc.vector.tensor_scalar(fx[s], fx[s], 0.0, 1.0, AL.max, AL.min)
            # flat gather index (pass-relative) + export for the wrap
            nc.vector.scalar_tensor_tensor(idxf[s], y0f[s], float(FC),
                                           x0f[s], AL.mult, AL.add)
            nc.vector.tensor_copy(out=idx16[s], in_=idxf[s])
            # export to DRAM permuted: d_scr[pp][t][r][qq][c]
            for qq in range(2):
                row = (2 * pp + qq) * 32
                nc.sync.dma_start(
                    out=d_scr[pp, :, :, qq, :],
                    in_=idx16[row: row + 18, :]
                    .rearrange("p (r c) -> p r c", r=8))
            # wrap: idxw[16g + 2r + qq, c] = idx16[row, r*128+c];
            # one stride-0 broadcast DMA per (k, dd)
            for k in range(9):
                for dd in range(2):
                    src = d_scr[pp, 2 * k + dd: 2 * k + dd + 1] \
                        .rearrange("t r q c -> t (r q c)") \
                        .to_broadcast([4, 2048])
                    nc.sync.dma_start(
                        out=idxw_t[pp][k][dd * 64:(dd + 1) * 64, 0:128],
                        in_=src)
                # y1 = y0 + FC, right after this k's wraps
                nc.vector.tensor_scalar(idxw_t[pp][k][:, 128:256],
                                        idxw_t[pp][k][:, 0:128],
                                        FC, None, AL.add)
            # corner weights (mask folded in): cw[:, yc, :, xc]
            nc.vector.tensor_tensor(u1[s], m_t[s], fy[s], AL.mult)
            nc.vector.tensor_tensor(u0[s], m_t[s], u1[s], AL.subtract)
            nc.vector.tensor_tensor(cw[s, 0, :, 1], u0[s], fx[s], AL.mult)
            nc.vector.tensor_tensor(cw[s, 0, :, 0], u0[s],
                                    cw[s, 0, :, 1], AL.subtract)
            nc.vector.tensor_tensor(cw[s, 1, :, 1], u1[s], fx[s], AL.mult)
            nc.vector.tensor_tensor(cw[s, 1, :, 0], u1[s],
                                    cw[s, 1, :, 1], AL.subtract)

        with tc.tile_pool(name="ph_a", bufs=1) as pa, \
             tc.tile_pool(name="psum_a", bufs=2, space="PSUM") as psa, \
             tc.tile_pool(name="psum_b", bufs=2, space="PSUM") as psb:
            x_pad = pa.tile([Ci, XR, XC], F16)
            feat1 = pa.tile([P, F1R, F1C], F16)
            l1 = pa.tile([Ci, 9, P], F16)
            l2 = pa.tile([P, 9, P], F16)
            lsc = pa.tile([Ci, P], F16)
            rm1 = pa.tile([P, F1R], F32)
            rmf = pa.tile([P, FR], F32)

            nc.sync.dma_start(out=l1[:], in_=d_l1[:])
            for i in range(4):
                nc.sync.dma_start(out=x_pad[:, 3 * i: 3 * i + 3, :],
                                  in_=d_x[:, 3 * i: 3 * i + 3, :])
            for i in range(7):
                r0, r1 = 12 + 10 * i, min(12 + 10 * (i + 1), XR)
                nc.sync.dma_start(out=x_pad[:, r0:r1, :],
                                  in_=d_x[:, r0:r1, :])
            for t, dref in ((l2, d_l2), (lsc, d_lsc),
                            (rm1, d_rm1), (rmf, d_rmf), (loff, d_loff),
                            (bq, d_bq), (basey, d_by), (basex, d_bx)):
                nc.sync.dma_start(out=t[:], in_=dref[:])

            nc.vector.memset(feat1[:, :, 0:1], 0.0)
            nc.vector.memset(feat1[:, :, F1C - 1:F1C], 0.0)

            def conv1_chunk(cki):
                # feat1 row f1 <-> global h0-4+f1; x_pad rows 2*f1+ty
                r0 = cki * 4
                ps = psa.tile([P, 4, W], F32, tag="ps1")
                for t in range(9):
                    ty, tx = t // 3, t % 3
                    rhs = x_pad[:, 2 * r0 + ty: 2 * r0 + ty + 7: 2,
                                tx: tx + 2 * W - 1: 2]
                    nc.tensor.matmul(ps[:], l1[:, t, :], rhs,
                                     start=(t == 0), stop=(t == 8))
                nc.scalar.activation(feat1[:, r0:r0 + 4, 1:1 + W], ps[:],
                                     AF.Relu, bias=beta1, scale=inv1)
                nc.vector.tensor_tensor(
                    feat1[:, r0:r0 + 4, :], feat1[:, r0:r0 + 4, :],
                    rm1[:, r0:r0 + 4, None].to_broadcast([P, 4, F1C]),
                    AL.mult)

            def conv2_chunk(cki):
                # F row f2 <-> global h0-3+f2; feat1 rows f2+ty
                r0 = cki * 4
                nrow = min(4, FR - r0)
                if cki <= 5:
                    fdt, fdt4, rb = fd_a, fda4, r0
                else:
                    fdt, fdt4, rb = fd_b, fdb4, r0 - 16
                ps = psa.tile([P, 4, W], F32, tag="ps2")
                for t in range(9):
                    ty, tx = t // 3, t % 3
                    rhs = feat1[:, r0 + ty: r0 + ty + nrow, tx: tx + W]
                    nc.tensor.matmul(ps[:, :nrow], l2[:, t, :], rhs,
                                     start=(t == 0), stop=False)
                rhs_sc = x_pad[:, 2 * r0 + 3: 2 * r0 + 2 + 2 * nrow: 2,
                               1: 2 * W: 2]
                nc.tensor.matmul(ps[:, :nrow], lsc[:], rhs_sc,
                                 start=False, stop=True)
                nc.scalar.activation(fdt4[:, rb:rb + nrow, 3:3 + W, 0],
                                     ps[:, :nrow], AF.Relu,
                                     bias=beta2, scale=inv2)
                nc.vector.tensor_tensor(
                    fdt4[:, rb:rb + nrow, :, 0], fdt4[:, rb:rb + nrow, :, 0],
                    rmf[:, r0:r0 + nrow, None].to_broadcast([P, nrow, FC]),
                    AL.mult)
                a0, a1 = rb * FC, (rb + nrow) * FC
                nc.vector.tensor_copy(out=fdt[:, max(a0 - 1, 0):a1 - 1, 1],
                                      in_=fdt[:, max(a0, 1):a1, 0])
                if cki == 5:
                    # seed fd_b rows 0..7 (global 16..23) from fd_a; chunk
                    # 6's dup then fixes the row-7/col-133 pair boundary
                    nc.vector.tensor_copy(out=fd_b[:, 0:8 * FC, :],
                                          in_=fd_a[:, 16 * FC:24 * FC, :])

            def off_chunk(cki):
                # offset conv om: rows quant*18+k*2+d; out rows h0..h0+31
                r0 = cki * 4
                pq, c2 = cki // 2, cki % 2
                if cki <= 3:
                    fdt4, rb = fda4, r0
                else:
                    fdt4, rb = fdb4, r0 - 16
                ps = psb.tile([54, 4, W], F32)
                for t in range(9):
                    ty, tx = t // 3, t % 3
                    rhs = fdt4[:, rb + 2 + ty: rb + 6 + ty,
                               2 + tx: 2 + tx + W, 0]
                    nc.tensor.matmul(ps[:], loff[:, t, :], rhs,
                                     start=(t == 0), stop=(t == 8))
                om_sb = pbt.tile([54, 512], F32, tag="om_sb")
                nc.scalar.copy(om_sb[:], ps[:].rearrange("p a b -> p (a b)"))
                for q in range(3):
                    nc.sync.dma_start(
                        out=q_t[pq * 32: pq * 32 + 18, q,
                                c2 * 512:(c2 + 1) * 512],
                        in_=om_sb[q * 18:(q + 1) * 18, :])


            for c in range(4):
                conv1_chunk(c)
            for j in range(10):
                if j + 4 <= 9:
                    conv1_chunk(j + 4)
                conv2_chunk(j)
                if j >= 2:
                    off_chunk(j - 2)
                if j == 5:
                    phase_b_pass(0)
            nc.vector.memset(fd_b[:, 22 * FC - 1: 22 * FC, 1], 0.0)

        # ================= Phase C: gather + Hadamard + einsum ==============
        with tc.tile_pool(name="psum_wb", bufs=4, space="PSUM") as psum_wb, \
             tc.tile_pool(name="psum_out", bufs=1, space="PSUM") as psum_out, \
             tc.tile_pool(name="gb", bufs=3) as gb_pool, \
             tc.tile_pool(name="wbs", bufs=2) as wbs_pool, \
             tc.tile_pool(name="rts", bufs=1) as rt_pool, \
             tc.tile_pool(name="outs", bufs=1) as out_pool:
            def pass_c(pp):
                pos = [psum_out.tile([P, NCHUNK], F32, name=f"pos{pp}_{qq}",
                                     tag=f"pos{qq}") for qq in range(2)]
                for k in range(9):
                    # gather x-pairs as single int32 elements: halves the
                    # per-gather element count vs d=2 fp16 (the tile itself
                    # stays f16 so the Hadamard keeps the 4x DVE mode)
                    g = gb_pool.tile([P, 4096, 2], F16)
                    fdt = fd_a if pp == 0 else fd_b
                    nc.gpsimd.ap_gather(
                        g[:].bitcast(I32), fdt[:].bitcast(I32)[:, 0:INW],
                        idxw_t[pp][k][:], channels=P,
                        num_elems=INW, d=1, num_idxs=4096)
                    g16 = g[:].rearrange("p a b -> p (a b)")
                    # wbs layout: (yc, c, r_loc, qq, xc)
                    wbs = wbs_pool.tile([P, 2, 128, 8, 2, 2], F16)
                    nj2 = 0
                    for yc in range(2):
                        for xc in range(2):
                            for qq in range(2):
                                rowp = (2 * pp + qq) * 32
                                # rhs in (c, r_loc) order -> psum col c*8+r
                                rhs = cw[rowp: rowp + 18, yc, :, xc] \
                                    .rearrange("p (r c) -> p c r", r=8)
                                for h5 in range(2):
                                    wb = psum_wb.tile([P, 512], F32,
                                                      tag="wb")
                                    nc.tensor.matmul(
                                        wb[:],
                                        esel[rowp: rowp + 18, k, :],
                                        rhs[:, h5 * 64:(h5 + 1) * 64, :],
                                        start=True, stop=True,
                                        tile_position=(rowp, 0))
                                    dst = wbs[:, yc,
                                              h5 * 64:(h5 + 1) * 64, :,
                                              qq, xc]
                                    wbv = wb[:].rearrange(
                                        "p (c r) -> p c r", c=64)
                                    if nj2 % 4 == 3:
                                        # balance: DVE takes 4 of 16 copies
                                        nc.vector.tensor_copy(out=dst,
                                                              in_=wbv)
                                    else:
                                        nc.scalar.copy(dst, wbv)
                                    nj2 += 1
                    rt = rt_pool.tile([P, 2, 128, 8, 2, 2], F16)
                    for yc in range(2):
                        nc.vector.tensor_tensor(
                            rt[:, yc].rearrange("p b c d e -> p (b c d e)"),
                            wbs[:, yc].rearrange("p b c d e -> p (b c d e)"),
                            g16[:, yc * 4096:(yc + 1) * 4096],
                            AL.mult)
                    for yc in range(2):
                        for xc in range(2):
                            for qq in range(2):
                                rhs = rt[:, yc, :, :, qq, xc]
                                for h5 in range(2):
                                    nc.tensor.matmul(
                                        pos[qq][:, h5 * 512:(h5 + 1) * 512],
                                        ldcn[:, k, :],
                                        rhs[:, h5 * 64:(h5 + 1) * 64, :],
                                        start=(k == 0 and yc == 0
                                               and xc == 0),
                                        stop=(k == 8 and yc == 1
                                              and xc == 1))
                ob = out_pool.tile([P, 2, NCHUNK], F32)
                for qq in range(2):
                    nc.scalar.activation(ob[:, qq, :], pos[qq][:], AF.Relu,
                                         bias=beta3, scale=inv3)
                nc.sync.dma_start(out=d_out[:, pp, :, :], in_=ob[:])

            pass_c(0)
            # pass-1 index/weight prep emitted AFTER pass-0's gathers so
            # its wrap DMAs land behind them on the DMA rings
            phase_b_pass(1)
            pass_c(1)

    nc.compile()
    return nc


_CACHE = {}


def _prep(inputs):
    f = {k: _f(v) for k, v in inputs.items()}
    inv1 = f['g1'] / np.sqrt(f['v1'] + EPS)
    beta1 = f['b1'] - f['m1'] * inv1
    inv2 = f['g2'] / np.sqrt(f['v2'] + EPS)
    beta2 = f['b2'] - f['m2'] * inv2
    invd = f['gd'] / np.sqrt(f['vd'] + EPS)
    betad = f['bd'] - f['md'] * invd
    inv3 = f['g3'] / np.sqrt(f['v3'] + EPS)
    beta3 = f['b3'] - f['m3'] * inv3

    lhsT1 = np.transpose(f['w1'], (1, 2, 3, 0)).reshape(Ci, 9, P)
    lhsT2 = np.transpose(f['w2'], (1, 2, 3, 0)).reshape(P, 9, P)
    wd = f['wd'][:, :, 0, 0] * (invd / inv2)[:, None]
    lhsT_sc = np.ascontiguousarray(wd.T)

    # offset conv rows: quant*18 + k*2 + d  <-  orig quant*18 + d*9 + k
    perm = np.zeros(54, dtype=np.int64)
    for quant in range(3):
        for kk in range(9):
            for dd in range(2):
                perm[quant * 18 + kk * 2 + dd] = quant * 18 + dd * 9 + kk
    ow = f['off_w'][perm]
    obias = f['off_b'][perm]
    lhsT_off = np.transpose(ow, (1, 2, 3, 0)).reshape(P, 9, 54)

    wr = f['dcn_w'].reshape(Co, DG, Cg, 9)
    lhsT_dcn = np.transpose(wr, (1, 2, 3, 0)).reshape(P, 9, Co)

    esel = np.zeros((P, 9, P), dtype=np.float32)
    for s in range(4):
        for kk in range(9):
            for dd in range(2):
                esel[32 * s + 2 * kk + dd, kk, dd * 64:(dd + 1) * 64] = 1.0

    cst = np.zeros((P, 8), dtype=np.float32)
    cst[:, 0], cst[:, 1] = inv1, beta1
    cst[:, 2], cst[:, 3] = inv2, beta2 + betad
    cst[:, 4], cst[:, 5] = inv3, beta3 + inv3 * f['dcn_b']

    bias_q = np.zeros((P, 3), dtype=np.float32)
    for pq in range(4):
        for kk in range(9):
            for dd in range(2):
                r = pq * 32 + kk * 2 + dd
                for quant in range(3):
                    bias_q[r, quant] = obias[quant * 18 + kk * 2 + dd]

    # coordinate base maps (pass-relative row base, offset bias folded in)
    base_y = np.zeros((P, NCHUNK), dtype=np.float32)
    base_x = np.zeros((P, NCHUNK), dtype=np.float32)
    px = np.arange(NCHUNK)
    for p in range(P):
        pq, t = p // 32, p % 32
        if t >= 18:
            continue
        kk, dd = t // 2, t % 2
        ky, kx = kk // 3, kk % 3
        base_y[p] = 8 * (pq % 2) + px // 128 + ky + 2 + bias_q[p, 0]
        base_x[p] = px % 128 + kx + 2 + bias_q[p, 1]

    return dict(
        lhsT1=_h(lhsT1), lhsT2=_h(lhsT2), lhsT_sc=_h(lhsT_sc),
        lhsT_off=_h(lhsT_off), lhsT_dcn=_h(lhsT_dcn), e_sel=_h(esel),
        consts=_f(cst), bias_q=_f(bias_q), base_y=base_y, base_x=base_x,
        x=f['x'])


def kernel(**inputs):
    cfg = _prep(inputs)
    x = cfg.pop('x')
    B = x.shape[0]

    if 'nc' not in _CACHE:
        _CACHE['nc'] = build_nc()
    nc = _CACHE['nc']

    in_maps = []
    for cid in range(8):
        b, q = cid // 4, cid % 4
        h0 = 32 * q
        xp = np.zeros((Ci, XR, XC), dtype=np.float16)
        r_lo = 2 * h0 - 9
        s_lo, s_hi = max(r_lo, 0), min(2 * h0 + 72, 256)
        xp[:, s_lo - r_lo: s_hi - r_lo, 1:257] = \
            x[b, :, s_lo:s_hi, :].astype(np.float16)
        rm1 = np.zeros((P, F1R), dtype=np.float32)
        for f1 in range(F1R):
            rm1[:, f1] = 1.0 if 0 <= h0 - 4 + f1 < H else 0.0
        rmf = np.zeros((P, FR), dtype=np.float32)
        for f2 in range(FR):
            rmf[:, f2] = 1.0 if 0 <= h0 - 3 + f2 < H else 0.0
        m = dict(cfg)
        m['x_shard'] = np.ascontiguousarray(xp)
        m['rowmask1'] = rm1
        m['rowmaskF'] = rmf
        in_maps.append(m)

    res = run_bass_kernel_spmd(nc, in_maps, core_ids=list(range(8)))
    out = np.zeros((B, Co, H, W), dtype=np.float32)
    for cid in range(8):
        b, q = cid // 4, cid % 4
        o = res.results[cid]['out']            # [P, pp, qq, m=c*8+r_loc]
        o = o.reshape(P, 2, 2, 128, 8)         # [P, pp, qq, c, r]
        o = np.transpose(o, (0, 1, 2, 4, 3))   # [P, pp, qq, r, c]
        out[b, :, 32 * q:32 * q + 32, :] = o.reshape(P, 32, 128)
    return out
